# revision 2
# baseline (speedup 1.0000x reference)
"""MultiHeadAttention (B=2,N=2048,C=1024,H=16,Dk=64) on 8 TRN2 cores.

Head-tensor-parallel: core c owns heads {2c,2c+1} for both batches.
Device computes qkv^T = Wqkv_s^T @ x^T, causal softmax(q k^T/8) @ v, and the
partial out-projection (rows 128c:128c+128 of W_out); host sums the 8
partials (the "all-reduce"), transposes, and adds the fused bias.
b_k drops (softmax shift invariance); b_v folds into the output bias.

v2 layout: input DMA chunked (n, kc) and overlapped with a n-outer phase A;
phase C (normalize + out-projection) fused per-t into the attention stream
with per-t reciprocals read straight from PSUM (no Dekker split, no
den16 gather DMAs); all PSUM evacuations on DVE so ACT only runs exps.
"""
import sys

sys.path.insert(0, "/opt/trn_rl_repo")
import numpy as np
import ml_dtypes
import concourse.bass as bass
import concourse.mybir as mybir
from concourse.bass_utils import run_bass_kernel_spmd
from concourse.tile import TileContext

F32 = mybir.dt.float32
F16 = mybir.dt.float16
BF16 = mybir.dt.bfloat16
AF = mybir.ActivationFunctionType
BF = ml_dtypes.bfloat16

T = 4096  # total tokens (2 batches x 2048)
TRACE = False
LAST_EXEC_NS = None
LAST_MEAN_NS = None

_MAX_WAITS = 1  # this neuronxcc build rejects instructions with more sem waits


def _split_excess_waits(nc, limit=_MAX_WAITS):
    """Move excess sem waits onto same-engine nops inserted just before the
    over-subscribed instruction (waits-before-inst on the same queue is
    semantically identical)."""
    ifaces = [nc.tensor, nc.scalar, nc.vector, nc.gpsimd, nc.sync]
    eng_map = {iface.engine: iface for iface in ifaces}
    f = nc.m.functions[0]
    for bb in list(f.blocks):
        il = bb.instructions
        i = 0
        while i < len(il):
            ins = il[i]
            si = ins.sync_info
            waits = list(si.on_wait) if si is not None else []
            if len(waits) > limit:
                keep = waits[-limit:]
                rest = waits[:-limit]
                ins.sync_info = mybir.SyncInfo(
                    on_wait=keep, on_update=list(si.on_update)
                )
                nops = []
                for k in range(0, len(rest), limit):
                    nop = eng_map[ins.engine].nop(nofuse=True)
                    nop.ins.sync_info = mybir.SyncInfo(
                        on_wait=rest[k : k + limit], on_update=[]
                    )
                    nops.append(nop.ins)
                for ni in nops:
                    for bb2 in list(f.blocks):
                        try:
                            bb2.instructions.remove(ni)
                            break
                        except ValueError:
                            pass
                for off, ni in enumerate(nops):
                    il.insert(i + off, ni)
                i += len(nops)
            i += 1


def _build():
    nc = bass.Bass("TRN2", target_bir_lowering=False, debug=False, num_devices=8)
    xt_d = nc.declare_dram_parameter("xt", (1024, T), BF16, isOutput=False)
    wqkv_d = nc.declare_dram_parameter("wqkv", (1024, 384), BF16, isOutput=False)
    bq_d = nc.declare_dram_parameter("bq", (128, 1), F32, isOutput=False)
    wout_d = nc.declare_dram_parameter("wout", (128, 1024), BF16, isOutput=False)
    tri_d = nc.declare_dram_parameter("tri", (128, 128), BF16, isOutput=False)
    s21_d = nc.declare_dram_parameter("s21", (1, 128), BF16, isOutput=False)
    s22_d = nc.declare_dram_parameter("s22", (1, 128), BF16, isOutput=False)
    ident_d = nc.declare_dram_parameter("ident", (128, 128), BF16, isOutput=False)
    outp_d = nc.declare_dram_parameter("outp", (1024, T), F16, isOutput=True)

    with TileContext(nc) as tc:
        with tc.tile_pool(name="sb", bufs=1) as sb:
            # ---- persistent tiles ----
            wq_t = [
                sb.tile((128, 384), BF16, tag=f"wq{kc}", name=f"wq{kc}")
                for kc in range(8)
            ]
            # per-(kc, n) chunk tiles so deps stay chunk-granular
            xt_t = [
                [
                    sb.tile((128, 512), BF16, tag=f"xt{kc}_{n}", name=f"xt{kc}_{n}")
                    for n in range(8)
                ]
                for kc in range(8)
            ]
            bq_t = sb.tile((128, 1), F32, tag="bq")
            wout_t = sb.tile((128, 1024), BF16, tag="wout")
            tri_t = sb.tile((128, 128), BF16, tag="tri")
            id_t = sb.tile((128, 128), BF16, tag="ident")
            s21_t = sb.tile((1, 128), BF16, tag="s21")
            s22_t = sb.tile((1, 128), BF16, tag="s22")

            # ---- input DMAs: small stuff + weights first, xt chunks in
            # consumption order (n, kc), alternating the two queues ----
            nc.gpsimd.dma_start(bq_t[:], bq_d[:, :])
            nc.gpsimd.dma_start(wout_t[:], wout_d[:, :])
            nc.gpsimd.dma_start(tri_t[:], tri_d[:, :])
            nc.gpsimd.dma_start(id_t[:], ident_d[:, :])
            nc.gpsimd.dma_start(s21_t[:], s21_d[:, :])
            nc.gpsimd.dma_start(s22_t[:], s22_d[:, :])
            for kc in range(8):
                eng = nc.sync if kc % 2 == 0 else nc.gpsimd
                eng.dma_start(wq_t[kc][:], wqkv_d[128 * kc : 128 * kc + 128, :])
            qi = 0
            for n in range(8):
                for kc in range(8):
                    eng = nc.sync if qi % 2 == 0 else nc.gpsimd
                    qi += 1
                    eng.dma_start(
                        xt_t[kc][n][:],
                        xt_d[128 * kc : 128 * kc + 128, 512 * n : 512 * n + 512],
                    )

            q_T = sb.tile((128, T), BF16, tag="q_T")
            k_T = sb.tile((128, T), BF16, tag="k_T")
            v_T = sb.tile((128, T), BF16, tag="v_T")
            vext = [
                sb.tile((128, 2080), BF16, tag=f"vext{b}", name=f"vext{b}")
                for b in range(2)
            ]
            nc.vector.memset(vext[0][:], 1.0)
            nc.vector.memset(vext[1][:], 1.0)

            # es buffers for diagonal key-blocks: fully-masked columns are
            # zeroed once and never rewritten (exp writes only unmasked cols)
            es_diag = [
                sb.tile((128, 1024), BF16, tag=f"esd{r}", name=f"esd{r}")
                for r in range(4)
            ]
            for r in range(1, 4):
                nc.gpsimd.memset(es_diag[r][:, 0 : 128 * r], 0.0)
                nc.gpsimd.memset(es_diag[r][:, 512 : 512 + 128 * r], 0.0)

            # ---- phase A (+V): qkv^T chunk-pipelined against the xt DMAs,
            # v transposed into vext right after each chunk's v lands ----
            dst = [q_T, k_T, v_T]
            with tc.tile_pool(name="psA", bufs=1, space="PSUM") as psA:
                for n in range(8):
                    for m in range(3):
                        ps = psA.tile((128, 512), F32, tag="a", bufs=3)
                        for kc in range(8):
                            nc.tensor.matmul(
                                ps[:],
                                wq_t[kc][:, 128 * m : 128 * m + 128],
                                xt_t[kc][n][:],
                                start=(kc == 0),
                                stop=(kc == 7),
                            )
                        o = dst[m][:, 512 * n : 512 * n + 512]
                        if m == 0:
                            nc.vector.tensor_scalar_add(o, ps[:], bq_t[:, 0:1])
                        else:
                            nc.vector.tensor_copy(o, ps[:])
                    for w in range(4):
                        tglob = 4 * n + w
                        b, jj = divmod(tglob, 16)
                        trp = psA.tile((128, 128), BF16, tag="trp", bufs=2)
                        nc.tensor.transpose(
                            trp[:], v_T[:, 128 * tglob : 128 * tglob + 128], id_t[:]
                        )
                        # single DVE copy lands both halves: out chunks at
                        # 65*jj and 65*(16+jj) (stride 1040)
                        c0 = 65 * jj
                        oslc = vext[b][:, c0 : c0 + 64]
                        islc = trp[:]
                        o_ap = bass.AP(
                            oslc.tensor,
                            oslc.offset,
                            [[oslc.ap[0][0], oslc.ap[0][1]], [1040, 2], [1, 64]],
                        )
                        i_ap = bass.AP(
                            islc.tensor,
                            islc.offset,
                            [[islc.ap[0][0], islc.ap[0][1]], [64, 2], [1, 64]],
                        )
                        nc.vector.tensor_copy(o_ap, i_ap)

            # ---- fused phase B+C: causal attention, then per-t normalize +
            # partial out-projection, so out-proj matmuls/copies/DMAs overlap
            # the next t's ACT-bound exp stream ----
            with tc.tile_pool(name="psB", bufs=1, space="PSUM") as psB:
                for t in range(8):
                    b, i = divmod(t, 4)
                    nj = 4 * i + 4
                    qs = 2048 * b + 512 * i
                    av = psB.tile((65, 1024), F32, tag="av", bufs=1)
                    for jj in range(nj):
                        sps = psB.tile((128, 1024), F32, tag="sps", bufs=2)
                        ks = 2048 * b + 128 * jj
                        for hl in range(2):
                            nc.tensor.matmul(
                                sps[:, 512 * hl : 512 * hl + 512],
                                k_T[64 * hl : 64 * hl + 64, ks : ks + 128],
                                q_T[64 * hl : 64 * hl + 64, qs : qs + 512],
                                start=True,
                                stop=True,
                                skip_group_check=True,
                            )
                        r = jj - 4 * i
                        if r < 0:
                            es = sb.tile((128, 1024), BF16, tag="es", bufs=3)
                            nc.scalar.activation(es[:], sps[:], AF.Exp, scale=0.125)
                        else:
                            es = es_diag[r]
                            if r == 0:
                                nc.scalar.activation(
                                    es[:], sps[:], AF.Exp, scale=0.125
                                )
                            else:
                                # one 2-chunk ACT call covers both hl halves
                                w = 512 - 128 * r
                                oslc = es[:, 128 * r : 128 * r + w]
                                islc = sps[:, 128 * r : 128 * r + w]
                                o_ap = bass.AP(
                                    oslc.tensor,
                                    oslc.offset,
                                    [
                                        [oslc.ap[0][0], oslc.ap[0][1]],
                                        [512, 2],
                                        [1, w],
                                    ],
                                )
                                i_ap = bass.AP(
                                    islc.tensor,
                                    islc.offset,
                                    [
                                        [islc.ap[0][0], islc.ap[0][1]],
                                        [512, 2],
                                        [1, w],
                                    ],
                                )
                                nc.scalar.activation(
                                    o_ap, i_ap, AF.Exp, scale=0.125
                                )
                            for hl in range(2):
                                c0 = 512 * hl + 128 * r
                                nc.vector.tensor_mul(
                                    es[:, c0 : c0 + 128],
                                    es[:, c0 : c0 + 128],
                                    tri_t[:],
                                )
                        for hl in range(2):
                            c = 65 * (16 * hl + jj)
                            nc.tensor.matmul(
                                av[:, 512 * hl : 512 * hl + 512],
                                vext[b][:, c : c + 65],
                                es[:, 512 * hl : 512 * hl + 512],
                                start=(jj == 0),
                                stop=(jj == nj - 1),
                                skip_group_check=True,
                            )
                    # per-t normalize: reciprocal of the denominator row
                    # straight from PSUM (64->0 partition shift is
                    # quadrant-aligned), broadcast via two K=1 matmuls
                    rec1 = sb.tile((1, 512), F32, tag="rec1", bufs=2)
                    rec2 = sb.tile((1, 512), F32, tag="rec2", bufs=2)
                    nc.vector.reciprocal(rec1[:], av[64:65, 0:512])
                    nc.vector.reciprocal(rec2[:], av[64:65, 512:1024])
                    rb1 = sb.tile((1, 512), BF16, tag="rb1", bufs=2)
                    rb2 = sb.tile((1, 512), BF16, tag="rb2", bufs=2)
                    nc.vector.tensor_copy(rb1[:], rec1[:])
                    nc.vector.tensor_copy(rb2[:], rec2[:])
                    bcp = psB.tile((128, 512), F32, tag="op", bufs=2)
                    nc.tensor.matmul(
                        bcp[:],
                        s21_t[:],
                        rb1[:],
                        start=True,
                        stop=False,
                        skip_group_check=True,
                    )
                    nc.tensor.matmul(
                        bcp[:],
                        s22_t[:],
                        rb2[:],
                        start=False,
                        stop=True,
                        skip_group_check=True,
                    )
                    bcps = sb.tile((128, 512), BF16, tag="bcps", bufs=2)
                    nc.vector.tensor_copy(bcps[:], bcp[:])
                    attnT = sb.tile((128, 512), BF16, tag="attnT", bufs=2)
                    for hl in range(2):
                        nc.vector.tensor_mul(
                            attnT[64 * hl : 64 * hl + 64, :],
                            av[0:64, 512 * hl : 512 * hl + 512],
                            bcps[64 * hl : 64 * hl + 64, :],
                        )
                    for mo in range(8):
                        op = psB.tile((128, 512), F32, tag="op", bufs=2)
                        nc.tensor.matmul(
                            op[:],
                            wout_t[:, 128 * mo : 128 * mo + 128],
                            attnT[:],
                            start=True,
                            stop=True,
                            skip_group_check=True,
                        )
                        osb = sb.tile((128, 512), F16, tag="osb", bufs=4)
                        nc.vector.tensor_copy(osb[:], op[:])
                        eng = nc.sync if mo % 2 == 0 else nc.gpsimd
                        eng.dma_start(
                            outp_d[128 * mo : 128 * mo + 128, qs : qs + 512],
                            osb[:],
                        )
    _split_excess_waits(nc)
    return nc


def kernel(**inputs):
    global LAST_EXEC_NS, LAST_MEAN_NS
    x = np.asarray(inputs["x"], np.float32)
    Wqkv = np.asarray(inputs["W_qkv"], np.float32)
    bqkv = np.asarray(inputs["b_qkv"], np.float32)
    Wout = np.asarray(inputs["W_out"], np.float32)
    bout = np.asarray(inputs["b_out"], np.float32)

    xt = np.ascontiguousarray(x.reshape(T, 1024).T).astype(BF)
    kk = np.arange(128)[:, None]
    qq = np.arange(128)[None, :]
    tri = (qq >= kk).astype(BF)
    ident = np.eye(128).astype(BF)
    s21 = np.zeros((1, 128), BF)
    s21[0, 0:64] = 1.0
    s22 = np.zeros((1, 128), BF)
    s22[0, 64:128] = 1.0

    in_maps = []
    for c in range(8):
        s = 128 * c
        wq = np.ascontiguousarray(
            np.concatenate(
                [
                    Wqkv[:, s : s + 128],
                    Wqkv[:, 1024 + s : 1024 + s + 128],
                    Wqkv[:, 2048 + s : 2048 + s + 128],
                ],
                axis=1,
            )
        ).astype(BF)
        in_maps.append(
            {
                "xt": xt,
                "wqkv": wq,
                "bq": np.ascontiguousarray(
                    bqkv[s : s + 128].reshape(128, 1)
                ).astype(np.float32),
                "wout": np.ascontiguousarray(Wout[s : s + 128, :]).astype(BF),
                "tri": tri,
                "s21": s21,
                "s22": s22,
                "ident": ident,
            }
        )

    nc = _build()
    res = run_bass_kernel_spmd(nc, in_maps, list(range(8)), trace=TRACE)
    LAST_EXEC_NS = res.exec_time_ns
    LAST_MEAN_NS = res.mean_exec_time_ns

    total = np.zeros((1024, T), np.float32)
    for c in range(8):
        total += np.asarray(res.results[c]["outp"]).astype(np.float32)
    beff = (
        bout.astype(np.float64) + bqkv[2048:].astype(np.float64) @ Wout.astype(np.float64)
    ).astype(np.float32)
    out = total.T.reshape(2, 2048, 1024) + beff
    return out.astype(np.float32)
